# revision 23
# baseline (speedup 1.0000x reference)
"""Causal self-attention (B=2, T=2048, D=1024, H=16) on 8 TRN2 NeuronCores.

Sharding: data-parallel over batch (cores 0-3 -> batch 0, cores 4-7 -> batch 1),
tensor-parallel over heads (4 heads / 256 output dims per core). Each core
computes q/k/v projections for its heads, causal flash-style attention, and a
partial output projection (contraction over its 256 dims of Wo). The host sums
the 4 partials per batch and adds bo.

Precision: fp8 noise on attention inputs is only harmful for EARLY tokens
(queries that average few keys get no noise suppression).  So queries/tokens
0..511 (tq=0) run fully in bf16, while tq>=1 uses fp8: projections as
fp8e4m3 DoubleRow matmuls (2 contraction tiles per instruction, 0.5 cyc/row),
S matmuls in fp8e3m4, attention*V in fp8e4m3 DoubleRow over key-block pairs.
fp8 weights are pre-scaled by 32 on the host (removed in the psum->sbuf
casts); exp() runs with bias=-2 so attention weights stay well inside e4m3
range (the softmax normalization cancels the constant).  The output
projection runs in bf16.  Validated full-output rel err ~6e-3.
"""
import sys

sys.path.insert(0, '/opt/trn_rl_repo')

import numpy as np

import concourse.bass as bass  # noqa: F401  (import keeps bass registered)
import concourse.mybir as mybir
import concourse.tile as tile
from concourse import bacc
from concourse.bass_utils import run_bass_kernel_spmd

F32 = mybir.dt.float32
F32R = mybir.dt.float32r
BF16 = mybir.dt.bfloat16
F8E4 = mybir.dt.float8e4   # e4m3
F8E3 = mybir.dt.float8e3   # e3m4
AF = mybir.ActivationFunctionType
ALU = mybir.AluOpType
DRM = mybir.MatmulPerfMode.DoubleRow

B, T, D, H, HD = 2, 2048, 1024, 16, 64
NCORES = 8
E = 256          # output dims per core (4 heads x 64)
DM = 8           # d_model chunks of 128
TQ = 512
NTQ = T // TQ    # 4
TL = T - TQ      # late tokens (fp8 region)
WS = 32.0        # host-side fp8 weight pre-scale (removed in casts)

_CACHE = {}


def _build():
    nc = bacc.Bacc("TRN2", target_bir_lowering=False, debug=False)

    xbf_d = nc.dram_tensor("xbf", [128, DM, TQ], BF16, kind="ExternalInput")
    xf8_d = nc.dram_tensor("xf8", [128, DM, TL], F8E4, kind="ExternalInput")
    wbf_d = {w: nc.dram_tensor(f"w{w}bf", [128, DM, E], BF16, kind="ExternalInput")
             for w in "qkv"}
    wf8_d = {w: nc.dram_tensor(f"w{w}f8", [128, DM // 2, 2, E], F8E4,
                               kind="ExternalInput")
             for w in "qkv"}
    wo_d = nc.dram_tensor("wo", [E, D], BF16, kind="ExternalInput")
    bq_d = nc.dram_tensor("bq", [E, 1], F32, kind="ExternalInput")
    bk_d = nc.dram_tensor("bk", [E, 1], F32, kind="ExternalInput")
    bvbf_d = nc.dram_tensor("bvbf", [1, E], BF16, kind="ExternalInput")
    bvf8_d = nc.dram_tensor("bvf8", [1, E], F8E4, kind="ExternalInput")
    onebf_d = nc.dram_tensor("onebf", [1, 128], BF16, kind="ExternalInput")
    onef8_d = nc.dram_tensor("onef8", [1, 128], F8E4, kind="ExternalInput")
    onesr_d = nc.dram_tensor("onesr", [33, HD], BF16, kind="ExternalInput")
    outT = nc.dram_tensor("outT", [D, T], BF16, kind="ExternalOutput")

    with tile.TileContext(nc) as tc, nc.allow_low_precision(reason="fp8/bf16 attn"):
        with (
            tc.tile_pool(name="persist", bufs=1) as pp,
            tc.tile_pool(name="es", bufs=3) as esp,
            tc.tile_pool(name="esd", bufs=4) as esd,
            tc.tile_pool(name="ostage", bufs=3) as op_pool,
            tc.tile_pool(name="small", bufs=1) as sm,
            tc.tile_pool(name="psum", bufs=2, space="PSUM") as ps,
        ):
            # ---- persistent SBUF tiles
            xbf = pp.tile([128, DM, TQ], BF16, tag="xbf", name="xbf")
            xf8 = pp.tile([128, DM, TL], F8E4, tag="xf8", name="xf8")
            wbf = {w: pp.tile([128, DM, E], BF16, tag=f"w{w}bf", name=f"w{w}bf")
                   for w in "qkv"}
            wf8 = {w: pp.tile([128, DM // 2, 2, E], F8E4, tag=f"w{w}f8",
                               name=f"w{w}f8")
                   for w in "qkv"}
            wo_sb = [pp.tile([128, D], BF16, tag=f"wo{d2}", name=f"wo{d2}")
                     for d2 in range(2)]
            bq_sb, bk_sb = [], []
            for e2 in range(2):
                t_ = pp.tile([128, 1], F32, tag=f"bq{e2}")
                nc.sync.dma_start(out=t_[:], in_=bq_d[e2 * 128:(e2 + 1) * 128, :])
                bq_sb.append(t_)
                t_ = pp.tile([128, 1], F32, tag=f"bk{e2}")
                nc.sync.dma_start(out=t_[:], in_=bk_d[e2 * 128:(e2 + 1) * 128, :])
                bk_sb.append(t_)
            bvbf = pp.tile([1, E], BF16, tag="bvbf")
            nc.sync.dma_start(out=bvbf[:], in_=bvbf_d[:, :])
            bvf8 = pp.tile([1, E], F8E4, tag="bvf8")
            nc.sync.dma_start(out=bvf8[:], in_=bvf8_d[:, :])
            onebf = pp.tile([1, 128], BF16, tag="onebf")
            nc.sync.dma_start(out=onebf[:], in_=onebf_d[:, :])
            onef8 = pp.tile([1, 128], F8E4, tag="onef8")
            nc.sync.dma_start(out=onef8[:], in_=onef8_d[:, :])
            onesr = pp.tile([33, HD], BF16, tag="onesr")
            nc.sync.dma_start(out=onesr[:], in_=onesr_d[:, :])
            nbias = pp.tile([128, 1], F32, tag="nbias")
            nc.vector.memset(nbias[:], -2.0)
            dn = pp.tile([33, TQ], F32, tag="dn")
            nc.vector.memset(dn[:], 1.0)

            # chunk-interleaved startup so the first projection chain
            # starts after ~3 small DMAs instead of the whole working set
            for c in range(DM):
                for w in "qk":
                    nc.sync.dma_start(out=wbf[w][:, c, :], in_=wbf_d[w][:, c, :])
                nc.sync.dma_start(out=xbf[:, c, :], in_=xbf_d[:, c, :])
            nc.sync.dma_start(out=wbf["v"][:], in_=wbf_d["v"][:, :, :])
            for w in "qkv":
                nc.sync.dma_start(out=wf8[w][:], in_=wf8_d[w][:, :, :, :])
            for tq in range(1, NTQ):
                lo = (tq - 1) * TQ
                for c in range(DM):
                    nc.sync.dma_start(out=xf8[:, c, lo:lo + TQ],
                                      in_=xf8_d[:, c, lo:lo + TQ])
            for d2 in range(2):
                nc.sync.dma_start(out=wo_sb[d2][:], in_=wo_d[d2 * 128:(d2 + 1) * 128, :])

            # q/k: bf16 tiles cover tokens 0..511, fp8e3 tiles cover all tokens
            # (early-key slices are needed by late queries).
            qT_bf = [pp.tile([128, TQ], BF16, tag=f"qbf{i}", name=f"qbf{i}")
                     for i in range(2)]
            kT_bf = [pp.tile([128, TQ], BF16, tag=f"kbf{i}", name=f"kbf{i}")
                     for i in range(2)]
            qT_f8 = [pp.tile([128, T], F8E4, tag=f"qf8{i}", name=f"qf8{i}")
                     for i in range(2)]
            kT_f8 = [pp.tile([128, T], F8E4, tag=f"kf8{i}", name=f"kf8{i}")
                     for i in range(2)]
            # 32-split layouts for DoubleRow S matmuls: [32, head, half, T]
            q32 = [pp.tile([32, 2, 2, T], F8E4, tag=f"q32{i}", name=f"q32{i}")
                   for i in range(2)]
            k32 = [pp.tile([32, 2, 2, T], F8E4, tag=f"k32{i}", name=f"k32{i}")
                   for i in range(2)]

            def remap32(dst, src, lo, hi):
                for h in range(2):
                    for i in range(2):
                        p0 = 64 * h + 32 * i
                        nc.sync.dma_start(out=dst[:, h, i, lo:hi],
                                          in_=src[p0:p0 + 32, lo:hi])

            # v pair tiles: [keys, pair-member, head, HD+ones-col] (all blocks)
            v_sb = [pp.tile([128, 2, 4, 68], F8E4, tag=f"v{i}", name=f"v{i}")
                    for i in range(8)]
            for i in range(8):
                nc.vector.memset(v_sb[i][:, :, :, HD:HD + 1], 1.0)
            # bf16 v for key blocks 0..3 (early queries)
            v_bf = [pp.tile([128, 4, HD + 1], BF16, tag=f"vb{i}", name=f"vb{i}")
                    for i in range(4)]
            for i in range(4):
                nc.vector.memset(v_bf[i][:, :, HD:HD + 1], 1.0)
            yT = [pp.tile([128, T], BF16, tag=f"yT{i}", name=f"yT{i}") for i in range(2)]

            def project_qk(tq):
                for (w, b_sb, dst_bf, dst_f8) in (
                        ("q", bq_sb, qT_bf, qT_f8), ("k", bk_sb, kT_bf, kT_f8)):
                    for e2 in range(2):
                        pt = ps.tile([128, 1024], F32, tag="S",
                                     name=f"pqk_{w}_{tq}_{e2}")
                        if tq == 0:
                            for c in range(DM):
                                nc.tensor.matmul(
                                    pt[:, 0:TQ],
                                    wbf[w][:, c, e2 * 128:(e2 + 1) * 128],
                                    xbf[:, c, :],
                                    start=(c == 0), stop=(c == DM - 1))
                            nc.vector.tensor_scalar_add(
                                out=dst_bf[e2][:, :],
                                in0=pt[:, 0:TQ], scalar1=b_sb[e2][:])
                            if w == "k":
                                nc.gpsimd.tensor_copy(
                                    out=dst_f8[e2][:, 0:TQ],
                                    in_=dst_bf[e2][:, :])
                                remap32(k32[e2], dst_f8[e2], 0, TQ)
                        else:
                            lo = (tq - 1) * TQ
                            for c in range(DM // 2):
                                nc.tensor.matmul(
                                    pt[:, 0:TQ],
                                    wf8[w][:, c, :, e2 * 128:(e2 + 1) * 128],
                                    xf8[:, 2 * c:2 * c + 2, lo:lo + TQ],
                                    start=(c == 0), stop=(c == DM // 2 - 1),
                                    perf_mode=DRM)
                            nc.vector.tensor_scalar(
                                out=dst_f8[e2][:, tq * TQ:(tq + 1) * TQ],
                                in0=pt[:, 0:TQ],
                                scalar1=1.0 / WS, scalar2=b_sb[e2][:],
                                op0=ALU.mult, op1=ALU.add)
                            dst32 = k32[e2] if w == "k" else q32[e2]
                            remap32(dst32, dst_f8[e2],
                                    tq * TQ, (tq + 1) * TQ)

            def project_v(t):
                pv = ps.tile([128, E], F32, tag="y", name=f"pv_{t}")
                if t < 4:
                    for c in range(DM):
                        nc.tensor.matmul(
                            pv[:],
                            xbf[:, c, t * 128:(t + 1) * 128],
                            wbf["v"][:, c, :],
                            start=(c == 0), stop=False)
                    nc.tensor.matmul(
                        pv[:], onebf[0:1, :], bvbf[0:1, :], start=False, stop=True)
                    nc.vector.tensor_copy(
                        out=v_bf[t][:, :, 0:HD],
                        in_=pv[:].rearrange("p (h d) -> p h d", h=4))
                    nc.gpsimd.tensor_copy(
                        out=v_sb[t // 2][:, t % 2, :, 0:HD],
                        in_=v_bf[t][:, :, 0:HD])
                else:
                    lo = t * 128 - TQ
                    for c in range(DM // 2):
                        nc.tensor.matmul(
                            pv[:],
                            xf8[:, 2 * c:2 * c + 2, lo:lo + 128],
                            wf8["v"][:, c, :, :],
                            start=(c == 0), stop=False, perf_mode=DRM)
                    nc.tensor.matmul(
                        pv[:], onef8[0:1, :], bvf8[0:1, :], start=False, stop=True)
                    nc.vector.tensor_scalar_mul(
                        out=v_sb[t // 2][:, t % 2, :, 0:HD],
                        in0=pv[:].rearrange("p (h d) -> p h d", h=4),
                        scalar1=1.0 / WS)

            oproj_queue = []

            def oproj_chain(tq_o, e8):
                pt = ps.tile([128, TQ], F32, tag="b", name=f"poc_{tq_o}_{e8}")
                for d2 in range(2):
                    nc.tensor.matmul(
                        pt[:, 0:TQ],
                        wo_sb[d2][:, e8 * 128:(e8 + 1) * 128],
                        yT[d2][:, tq_o * TQ:(tq_o + 1) * TQ],
                        start=(d2 == 0), stop=(d2 == 1))
                ot = op_pool.tile([128, TQ], BF16, tag="ostage", name=f"oto_{tq_o}_{e8}")
                if e8 % 4 == 0:
                    nc.scalar.copy(out=ot[:], in_=pt[:, 0:TQ])
                else:
                    nc.vector.tensor_copy(out=ot[:], in_=pt[:, 0:TQ])
                nc.sync.dma_start(
                    out=outT[e8 * 128:(e8 + 1) * 128, tq_o * TQ:(tq_o + 1) * TQ],
                    in_=ot[:])

            def pop_filler():
                if oproj_queue:
                    oproj_chain(*oproj_queue.pop(0))

            def attention(tq, pr):
                bf = (tq == 0)
                kt = kT_bf[pr] if bf else kT_f8[pr]
                qt = qT_bf[pr] if bf else qT_f8[pr]
                qof = 0 if bf else tq * TQ
                py_a = ps.tile([HD + 1, TQ], F32, tag="y", name=f"pya_{tq}_{pr}")
                py_b = ps.tile([HD + 1, TQ], F32, tag="y", name=f"pyb_{tq}_{pr}")
                py = (py_a, py_b)
                npair = 0 if bf else 2 * tq
                units = [("pair", i) for i in range(npair)] + \
                        [("diag", o) for o in range(4)]

                def s_unit(u):
                    kind, idx = u
                    if kind == "pair":
                        est = esp.tile([128, 2, 2, TQ], F8E4, tag="es",
                                       name=f"es_{tq}_{pr}_{idx}")
                        for j in range(2):
                            tk = 2 * idx + j
                            ps_s = ps.tile([128, 1024], F32, tag="S",
                                           name=f"ps_{tq}_{pr}_{tk}")
                            for h in range(2):
                                nc.tensor.matmul(
                                    ps_s[:, h * TQ:(h + 1) * TQ],
                                    k32[pr][:, h, :, tk * 128:(tk + 1) * 128],
                                    q32[pr][:, h, :, qof:qof + TQ],
                                    start=True, stop=True, perf_mode=DRM)
                            nc.scalar.activation(
                                est[:, j, :, :], ps_s[:], AF.Exp,
                                bias=nbias[:], scale=0.125)
                        return (est, 0)
                    # diagonal block: only columns >= c0 are live
                    o = idx
                    tk = 4 * tq + o
                    c0 = 128 * o
                    n = TQ - c0
                    edt = BF16 if bf else F8E4
                    ps_s = ps.tile([128, 1024], F32, tag="S",
                                   name=f"psd_{tq}_{pr}_{o}")
                    ps2 = ps_s[:].rearrange("p (h q) -> p h q", h=2)
                    for h in range(2):
                        if bf:
                            nc.tensor.matmul(
                                ps_s[:, h * TQ + c0:(h + 1) * TQ],
                                kt[64 * h:64 * h + 64, tk * 128:(tk + 1) * 128],
                                qt[64 * h:64 * h + 64, qof + c0:qof + TQ],
                                start=True, stop=True)
                        else:
                            nc.tensor.matmul(
                                ps_s[:, h * TQ + c0:(h + 1) * TQ],
                                k32[pr][:, h, :, tk * 128:(tk + 1) * 128],
                                q32[pr][:, h, :, qof + c0:qof + TQ],
                                start=True, stop=True, perf_mode=DRM)
                    es_t = esd.tile([128, 2, TQ], edt, tag="esd",
                                    name=f"esd_{tq}_{pr}_{o}")
                    nc.scalar.activation(
                        es_t[:, :, c0:TQ], ps2[:, :, c0:TQ], AF.Exp,
                        bias=nbias[:], scale=0.125)
                    nc.gpsimd.affine_select(
                        out=es_t[:, :, c0:c0 + 128],
                        in_=es_t[:, :, c0:c0 + 128],
                        compare_op=ALU.is_ge,
                        fill=0.0,
                        base=0,
                        pattern=[[0, 2], [1, 128]],
                        channel_multiplier=-1)
                    return (es_t, c0)

                def y_unit(u, es, c0):
                    kind, idx = u
                    if kind == "pair":
                        for h in range(2):
                            nc.tensor.matmul(
                                py[h][:, :],
                                v_sb[idx][:, :, 2 * pr + h, 0:HD + 1],
                                es[:, :, h, :],
                                start=(idx == 0), stop=False, perf_mode=DRM)
                    else:
                        tk = 4 * tq + idx
                        for h in range(2):
                            vt = (v_bf[tk][:, 2 * pr + h, :] if bf
                                  else v_sb[tk // 2][:, tk % 2, 2 * pr + h, 0:HD + 1])
                            nc.tensor.matmul(
                                py[h][:, c0:TQ],
                                vt,
                                es[:, h, c0:TQ],
                                start=(npair == 0 and idx == 0), stop=(idx == 3))

                prev = None
                for u in units:
                    cur = (u, s_unit(u))
                    if prev is not None:
                        y_unit(prev[0], *prev[1])
                    pop_filler()
                    prev = cur
                y_unit(prev[0], *prev[1])

                # softmax denominators -> reciprocal -> broadcast multiply
                nc.vector.tensor_copy(out=dn[0:1, :], in_=py_a[HD:HD + 1, :])
                nc.vector.tensor_copy(out=dn[32:33, :], in_=py_b[HD:HD + 1, :])
                rc32 = sm.tile([33, TQ], F32, tag="rc32")
                nc.vector.reciprocal_approx_fast(out=rc32[:, :], in_=dn[:, :])
                rc = sm.tile([33, TQ], BF16, tag="rc")
                nc.vector.tensor_copy(out=rc[:, :], in_=rc32[:, :])
                pb = ps.tile([128, TQ], F32, tag="b", name=f"pb_{tq}_{pr}")
                for i in range(2):
                    nc.tensor.matmul(
                        pb[64 * i:64 * i + 64, :], onesr[32 * i:32 * i + 1, :],
                        rc[32 * i:32 * i + 1, :],
                        start=True, stop=True)
                bc = sm.tile([128, TQ], F32, tag="bc")
                nc.vector.tensor_copy(out=bc[:], in_=pb[:])
                for (i, pyt) in ((0, py_a), (1, py_b)):
                    row0 = 64 * i
                    nc.vector.tensor_mul(
                        out=yT[pr][row0:row0 + 64, tq * TQ:(tq + 1) * TQ],
                        in0=pyt[0:HD, :], in1=bc[64 * i:64 * i + 64, :])

            # ---- main schedule
            for tq in range(NTQ):
                project_qk(tq)
                for t in range(4 * tq, 4 * tq + 4):
                    project_v(t)
                attention(tq, 0)
                attention(tq, 1)
                oproj_queue.extend((tq, e8) for e8 in range(8))
            while oproj_queue:
                oproj_chain(*oproj_queue.pop(0))

    nc.compile()
    return nc


def _get_nc():
    if 'nc' not in _CACHE:
        _CACHE['nc'] = _build()
    return _CACHE['nc']


def _make_in_maps(x, Wq, bq, Wk, bk, Wv, bv, Wo, bo):
    import ml_dtypes
    E4 = ml_dtypes.float8_e4m3
    BF = ml_dtypes.bfloat16

    x = np.asarray(x, dtype=np.float32)
    Wq = np.asarray(Wq, dtype=np.float32)
    Wk = np.asarray(Wk, dtype=np.float32)
    Wv = np.asarray(Wv, dtype=np.float32)
    Wo = np.asarray(Wo, dtype=np.float32)
    bq = np.asarray(bq, dtype=np.float32)
    bk = np.asarray(bk, dtype=np.float32)
    bv = np.asarray(bv, dtype=np.float32)

    onesr = np.ones((33, HD), dtype=BF)

    def wbf_layout(W, hs):
        wt = W[hs].T.astype(BF)                       # [1024, 256]
        return np.ascontiguousarray(
            wt.reshape(DM, 128, E).transpose(1, 0, 2))
    def wf8_layout(W, hs):
        wt = (W[hs].T * WS).astype(E4)
        return np.ascontiguousarray(
            wt.reshape(DM // 2, 2, 128, E).transpose(2, 0, 1, 3))

    in_maps = []
    for c in range(NCORES):
        b, g = divmod(c, 4)
        hs = slice(g * E, (g + 1) * E)
        xT = np.ascontiguousarray(x[b].T)             # [1024, 2048]
        x4 = xT.reshape(DM, 128, T).transpose(1, 0, 2)  # [128, 8, 2048]
        in_maps.append({
            "xbf": np.ascontiguousarray(x4[:, :, 0:TQ]).astype(BF),
            "xf8": np.ascontiguousarray(x4[:, :, TQ:]).astype(E4),
            "wqbf": wbf_layout(Wq, hs), "wqf8": wf8_layout(Wq, hs),
            "wkbf": wbf_layout(Wk, hs), "wkf8": wf8_layout(Wk, hs),
            "wvbf": wbf_layout(Wv, hs), "wvf8": wf8_layout(Wv, hs),
            "wo": np.ascontiguousarray(Wo[:, hs].T).astype(BF),
            "bq": np.ascontiguousarray(bq[hs].reshape(E, 1)),
            "bk": np.ascontiguousarray(bk[hs].reshape(E, 1)),
            "bvbf": bv[hs].reshape(1, E).astype(BF),
            "bvf8": (bv[hs].reshape(1, E) * WS).astype(E4),
            "onebf": np.ones((1, 128), dtype=BF),
            "onef8": np.ones((1, 128), dtype=E4),
            "onesr": onesr,
        })
    return in_maps


def kernel(x, Wq, bq, Wk, bk, Wv, bv, Wo, bo, _run_kwargs=None):
    nc = _get_nc()
    in_maps = _make_in_maps(x, Wq, bq, Wk, bk, Wv, bv, Wo, bo)
    last_err = None
    for _attempt in range(3):
        try:
            res = run_bass_kernel_spmd(nc, in_maps, core_ids=list(range(NCORES)),
                                       **(_run_kwargs or {}))
            break
        except Exception as e:  # transient NRT/device hiccups: retry
            last_err = e
            import time as _time
            _time.sleep(2.0)
    else:
        raise last_err
    bo = np.asarray(bo, dtype=np.float32)
    out = np.empty((B, T, D), dtype=np.float32)
    for b in range(B):
        acc = res.results[4 * b]["outT"].astype(np.float32)
        for g in range(1, 4):
            acc += res.results[4 * b + g]["outT"].astype(np.float32)
        out[b] = acc.T + bo
    if _run_kwargs:
        _CACHE['last_results'] = res
    return out


# revision 24
# speedup vs baseline: 1.1569x; 1.1569x over previous
"""Causal self-attention (B=2, T=2048, D=1024, H=16) on 8 TRN2 NeuronCores.

Sharding: data-parallel over batch (cores 0-3 -> batch 0, cores 4-7 -> batch 1),
tensor-parallel over heads (4 heads / 256 output dims per core). Each core
computes q/k/v projections for its heads, causal flash-style attention, and a
partial output projection (contraction over its 256 dims of Wo). The host sums
the 4 partials per batch and adds bo.

Precision: fp8 noise on attention inputs is only harmful for EARLY tokens
(queries that average few keys get no noise suppression).  So queries/tokens
0..511 (tq=0) run fully in bf16, while tq>=1 uses fp8: projections as
fp8e4m3 DoubleRow matmuls (2 contraction tiles per instruction, 0.5 cyc/row),
S matmuls in fp8e3m4, attention*V in fp8e4m3 DoubleRow over key-block pairs.
fp8 weights are pre-scaled by 32 on the host (removed in the psum->sbuf
casts); exp() runs with bias=-2 so attention weights stay well inside e4m3
range (the softmax normalization cancels the constant).  The output
projection runs in bf16.  Validated full-output rel err ~6e-3.
"""
import sys

sys.path.insert(0, '/opt/trn_rl_repo')

import numpy as np

import concourse.bass as bass  # noqa: F401  (import keeps bass registered)
import concourse.mybir as mybir
import concourse.tile as tile
from concourse import bacc
from concourse.bass_utils import run_bass_kernel_spmd

F32 = mybir.dt.float32
F32R = mybir.dt.float32r
BF16 = mybir.dt.bfloat16
F8E4 = mybir.dt.float8e4   # e4m3
F8E3 = mybir.dt.float8e3   # e3m4
AF = mybir.ActivationFunctionType
ALU = mybir.AluOpType
DRM = mybir.MatmulPerfMode.DoubleRow

B, T, D, H, HD = 2, 2048, 1024, 16, 64
NCORES = 8
E = 256          # output dims per core (4 heads x 64)
DM = 8           # d_model chunks of 128
TQ = 512
NTQ = T // TQ    # 4
TL = T - TQ      # late tokens (fp8 region)
WS = 32.0        # host-side fp8 weight pre-scale (removed in casts)

_CACHE = {}


def _build():
    nc = bacc.Bacc("TRN2", target_bir_lowering=False, debug=False)

    xbf_d = nc.dram_tensor("xbf", [128, DM, TQ], BF16, kind="ExternalInput")
    xf8_d = nc.dram_tensor("xf8", [128, DM, TL], F8E4, kind="ExternalInput")
    wbf_d = {w: nc.dram_tensor(f"w{w}bf", [128, DM, E], BF16, kind="ExternalInput")
             for w in "qkv"}
    wf8_d = {w: nc.dram_tensor(f"w{w}f8", [128, DM // 2, 2, E], F8E4,
                               kind="ExternalInput")
             for w in "qkv"}
    wo_d = nc.dram_tensor("wo", [E, D], BF16, kind="ExternalInput")
    bq_d = nc.dram_tensor("bq", [E, 1], F32, kind="ExternalInput")
    bk_d = nc.dram_tensor("bk", [E, 1], F32, kind="ExternalInput")
    bvbf_d = nc.dram_tensor("bvbf", [1, E], BF16, kind="ExternalInput")
    bvf8_d = nc.dram_tensor("bvf8", [1, E], F8E4, kind="ExternalInput")
    onebf_d = nc.dram_tensor("onebf", [1, 128], BF16, kind="ExternalInput")
    onef8_d = nc.dram_tensor("onef8", [1, 128], F8E4, kind="ExternalInput")
    onesr_d = nc.dram_tensor("onesr", [33, HD], BF16, kind="ExternalInput")
    outT = nc.dram_tensor("outT", [D, T], BF16, kind="ExternalOutput")

    with tile.TileContext(nc) as tc, nc.allow_low_precision(reason="fp8/bf16 attn"):
        with (
            tc.tile_pool(name="persist", bufs=1) as pp,
            tc.tile_pool(name="es", bufs=3) as esp,
            tc.tile_pool(name="esd", bufs=4) as esd,
            tc.tile_pool(name="ostage", bufs=3) as op_pool,
            tc.tile_pool(name="small", bufs=1) as sm,
            tc.tile_pool(name="psum", bufs=2, space="PSUM") as ps,
        ):
            # ---- persistent SBUF tiles
            xbf = pp.tile([128, DM, TQ], BF16, tag="xbf", name="xbf")
            xf8 = pp.tile([128, DM, TL], F8E4, tag="xf8", name="xf8")
            wbf = {w: pp.tile([128, DM, E], BF16, tag=f"w{w}bf", name=f"w{w}bf")
                   for w in "qkv"}
            wf8 = {w: pp.tile([128, DM // 2, 2, E], F8E4, tag=f"w{w}f8",
                               name=f"w{w}f8")
                   for w in "qkv"}
            wo_sb = [pp.tile([128, D], BF16, tag=f"wo{d2}", name=f"wo{d2}")
                     for d2 in range(2)]
            bq_sb, bk_sb = [], []
            for e2 in range(2):
                t_ = pp.tile([128, 1], F32, tag=f"bq{e2}")
                nc.sync.dma_start(out=t_[:], in_=bq_d[e2 * 128:(e2 + 1) * 128, :])
                bq_sb.append(t_)
                t_ = pp.tile([128, 1], F32, tag=f"bk{e2}")
                nc.sync.dma_start(out=t_[:], in_=bk_d[e2 * 128:(e2 + 1) * 128, :])
                bk_sb.append(t_)
            bvbf = pp.tile([1, E], BF16, tag="bvbf")
            nc.sync.dma_start(out=bvbf[:], in_=bvbf_d[:, :])
            bvf8 = pp.tile([1, E], F8E4, tag="bvf8")
            nc.sync.dma_start(out=bvf8[:], in_=bvf8_d[:, :])
            onebf = pp.tile([1, 128], BF16, tag="onebf")
            nc.sync.dma_start(out=onebf[:], in_=onebf_d[:, :])
            onef8 = pp.tile([1, 128], F8E4, tag="onef8")
            nc.sync.dma_start(out=onef8[:], in_=onef8_d[:, :])
            onesr = pp.tile([33, HD], BF16, tag="onesr")
            nc.sync.dma_start(out=onesr[:], in_=onesr_d[:, :])
            nbias = pp.tile([128, 1], F32, tag="nbias")
            nc.vector.memset(nbias[:], -2.0)
            dn = pp.tile([33, TQ], F32, tag="dn")
            nc.vector.memset(dn[:], 1.0)

            # chunk-interleaved startup so the first projection chain
            # starts after ~3 small DMAs instead of the whole working set
            for c in range(DM):
                for w in "qk":
                    nc.sync.dma_start(out=wbf[w][:, c, :], in_=wbf_d[w][:, c, :])
                nc.sync.dma_start(out=xbf[:, c, :], in_=xbf_d[:, c, :])
            nc.sync.dma_start(out=wbf["v"][:], in_=wbf_d["v"][:, :, :])
            for w in "qkv":
                nc.sync.dma_start(out=wf8[w][:], in_=wf8_d[w][:, :, :, :])
            for tq in range(1, NTQ):
                lo = (tq - 1) * TQ
                for c in range(DM):
                    nc.sync.dma_start(out=xf8[:, c, lo:lo + TQ],
                                      in_=xf8_d[:, c, lo:lo + TQ])
            for d2 in range(2):
                nc.sync.dma_start(out=wo_sb[d2][:], in_=wo_d[d2 * 128:(d2 + 1) * 128, :])

            # q/k: bf16 tiles cover tokens 0..511, fp8e3 tiles cover all tokens
            # (early-key slices are needed by late queries).
            qT_bf = [pp.tile([128, TQ], BF16, tag=f"qbf{i}", name=f"qbf{i}")
                     for i in range(2)]
            kT_bf = [pp.tile([128, TQ], BF16, tag=f"kbf{i}", name=f"kbf{i}")
                     for i in range(2)]
            qT_f8 = [pp.tile([128, T], F8E3, tag=f"qf8{i}", name=f"qf8{i}")
                     for i in range(2)]
            kT_f8 = [pp.tile([128, T], F8E3, tag=f"kf8{i}", name=f"kf8{i}")
                     for i in range(2)]
            # v pair tiles: [keys, pair-member, head, HD+ones-col] (all blocks)
            v_sb = [pp.tile([128, 2, 4, 68], F8E4, tag=f"v{i}", name=f"v{i}")
                    for i in range(8)]
            for i in range(8):
                nc.vector.memset(v_sb[i][:, :, :, HD:HD + 1], 1.0)
            # bf16 v for key blocks 0..3 (early queries)
            v_bf = [pp.tile([128, 4, HD + 1], BF16, tag=f"vb{i}", name=f"vb{i}")
                    for i in range(4)]
            for i in range(4):
                nc.vector.memset(v_bf[i][:, :, HD:HD + 1], 1.0)
            yT = [pp.tile([128, T], BF16, tag=f"yT{i}", name=f"yT{i}") for i in range(2)]

            def project_qk(tq):
                for (w, b_sb, dst_bf, dst_f8) in (
                        ("q", bq_sb, qT_bf, qT_f8), ("k", bk_sb, kT_bf, kT_f8)):
                    for e2 in range(2):
                        pt = ps.tile([128, 1024], F32, tag="S",
                                     name=f"pqk_{w}_{tq}_{e2}")
                        if tq == 0:
                            for c in range(DM):
                                nc.tensor.matmul(
                                    pt[:, 0:TQ],
                                    wbf[w][:, c, e2 * 128:(e2 + 1) * 128],
                                    xbf[:, c, :],
                                    start=(c == 0), stop=(c == DM - 1))
                            nc.vector.tensor_scalar_add(
                                out=dst_bf[e2][:, :],
                                in0=pt[:, 0:TQ], scalar1=b_sb[e2][:])
                            if w == "k":
                                nc.gpsimd.tensor_copy(
                                    out=dst_f8[e2][:, 0:TQ],
                                    in_=dst_bf[e2][:, :])
                        else:
                            lo = (tq - 1) * TQ
                            for c in range(DM // 2):
                                nc.tensor.matmul(
                                    pt[:, 0:TQ],
                                    wf8[w][:, c, :, e2 * 128:(e2 + 1) * 128],
                                    xf8[:, 2 * c:2 * c + 2, lo:lo + TQ],
                                    start=(c == 0), stop=(c == DM // 2 - 1),
                                    perf_mode=DRM)
                            nc.vector.tensor_scalar(
                                out=dst_f8[e2][:, tq * TQ:(tq + 1) * TQ],
                                in0=pt[:, 0:TQ],
                                scalar1=1.0 / WS, scalar2=b_sb[e2][:],
                                op0=ALU.mult, op1=ALU.add)

            def project_v(t):
                pv = ps.tile([128, E], F32, tag="y", name=f"pv_{t}")
                if t < 4:
                    for c in range(DM):
                        nc.tensor.matmul(
                            pv[:],
                            xbf[:, c, t * 128:(t + 1) * 128],
                            wbf["v"][:, c, :],
                            start=(c == 0), stop=False)
                    nc.tensor.matmul(
                        pv[:], onebf[0:1, :], bvbf[0:1, :], start=False, stop=True)
                    nc.vector.tensor_copy(
                        out=v_bf[t][:, :, 0:HD],
                        in_=pv[:].rearrange("p (h d) -> p h d", h=4))
                    nc.gpsimd.tensor_copy(
                        out=v_sb[t // 2][:, t % 2, :, 0:HD],
                        in_=v_bf[t][:, :, 0:HD])
                else:
                    lo = t * 128 - TQ
                    for c in range(DM // 2):
                        nc.tensor.matmul(
                            pv[:],
                            xf8[:, 2 * c:2 * c + 2, lo:lo + 128],
                            wf8["v"][:, c, :, :],
                            start=(c == 0), stop=False, perf_mode=DRM)
                    nc.tensor.matmul(
                        pv[:], onef8[0:1, :], bvf8[0:1, :], start=False, stop=True)
                    nc.vector.tensor_scalar_mul(
                        out=v_sb[t // 2][:, t % 2, :, 0:HD],
                        in0=pv[:].rearrange("p (h d) -> p h d", h=4),
                        scalar1=1.0 / WS)

            oproj_queue = []

            def oproj_chain(tq_o, e8):
                pt = ps.tile([128, TQ], F32, tag="b", name=f"poc_{tq_o}_{e8}")
                for d2 in range(2):
                    nc.tensor.matmul(
                        pt[:, 0:TQ],
                        wo_sb[d2][:, e8 * 128:(e8 + 1) * 128],
                        yT[d2][:, tq_o * TQ:(tq_o + 1) * TQ],
                        start=(d2 == 0), stop=(d2 == 1))
                ot = op_pool.tile([128, TQ], BF16, tag="ostage", name=f"oto_{tq_o}_{e8}")
                if e8 % 4 == 0:
                    nc.scalar.copy(out=ot[:], in_=pt[:, 0:TQ])
                else:
                    nc.vector.tensor_copy(out=ot[:], in_=pt[:, 0:TQ])
                nc.sync.dma_start(
                    out=outT[e8 * 128:(e8 + 1) * 128, tq_o * TQ:(tq_o + 1) * TQ],
                    in_=ot[:])

            def pop_filler():
                if oproj_queue:
                    oproj_chain(*oproj_queue.pop(0))

            def attention(tq, pr):
                bf = (tq == 0)
                kt = kT_bf[pr] if bf else kT_f8[pr]
                qt = qT_bf[pr] if bf else qT_f8[pr]
                qof = 0 if bf else tq * TQ
                py_a = ps.tile([HD + 1, TQ], F32, tag="y", name=f"pya_{tq}_{pr}")
                py_b = ps.tile([HD + 1, TQ], F32, tag="y", name=f"pyb_{tq}_{pr}")
                py = (py_a, py_b)
                npair = 0 if bf else 2 * tq
                units = [("pair", i) for i in range(npair)] + \
                        [("diag", o) for o in range(4)]

                def s_unit(u):
                    kind, idx = u
                    if kind == "pair":
                        est = esp.tile([128, 2, 2, TQ], F8E4, tag="es",
                                       name=f"es_{tq}_{pr}_{idx}")
                        for j in range(2):
                            tk = 2 * idx + j
                            ps_s = ps.tile([128, 1024], F32, tag="S",
                                           name=f"ps_{tq}_{pr}_{tk}")
                            for h in range(2):
                                nc.tensor.matmul(
                                    ps_s[:, h * TQ:(h + 1) * TQ],
                                    kt[64 * h:64 * h + 64, tk * 128:(tk + 1) * 128],
                                    qt[64 * h:64 * h + 64, qof:qof + TQ],
                                    start=True, stop=True)
                            nc.scalar.activation(
                                est[:, j, :, :], ps_s[:], AF.Exp,
                                bias=nbias[:], scale=0.125)
                        return (est, 0)
                    # diagonal block: only columns >= c0 are live
                    o = idx
                    tk = 4 * tq + o
                    c0 = 128 * o
                    n = TQ - c0
                    edt = BF16 if bf else F8E4
                    ps_s = ps.tile([128, 1024], F32, tag="S",
                                   name=f"psd_{tq}_{pr}_{o}")
                    ps2 = ps_s[:].rearrange("p (h q) -> p h q", h=2)
                    for h in range(2):
                        nc.tensor.matmul(
                            ps_s[:, h * TQ + c0:(h + 1) * TQ],
                            kt[64 * h:64 * h + 64, tk * 128:(tk + 1) * 128],
                            qt[64 * h:64 * h + 64, qof + c0:qof + TQ],
                            start=True, stop=True)
                    es_t = esd.tile([128, 2, TQ], edt, tag="esd",
                                    name=f"esd_{tq}_{pr}_{o}")
                    nc.scalar.activation(
                        es_t[:, :, c0:TQ], ps2[:, :, c0:TQ], AF.Exp,
                        bias=nbias[:], scale=0.125)
                    nc.gpsimd.affine_select(
                        out=es_t[:, :, c0:c0 + 128],
                        in_=es_t[:, :, c0:c0 + 128],
                        compare_op=ALU.is_ge,
                        fill=0.0,
                        base=0,
                        pattern=[[0, 2], [1, 128]],
                        channel_multiplier=-1)
                    return (es_t, c0)

                def y_unit(u, es, c0):
                    kind, idx = u
                    if kind == "pair":
                        for h in range(2):
                            nc.tensor.matmul(
                                py[h][:, :],
                                v_sb[idx][:, :, 2 * pr + h, 0:HD + 1],
                                es[:, :, h, :],
                                start=(idx == 0), stop=False, perf_mode=DRM)
                    else:
                        tk = 4 * tq + idx
                        for h in range(2):
                            vt = (v_bf[tk][:, 2 * pr + h, :] if bf
                                  else v_sb[tk // 2][:, tk % 2, 2 * pr + h, 0:HD + 1])
                            nc.tensor.matmul(
                                py[h][:, c0:TQ],
                                vt,
                                es[:, h, c0:TQ],
                                start=(npair == 0 and idx == 0), stop=(idx == 3))

                prev = None
                for u in units:
                    cur = (u, s_unit(u))
                    if prev is not None:
                        y_unit(prev[0], *prev[1])
                    pop_filler()
                    prev = cur
                y_unit(prev[0], *prev[1])

                # softmax denominators -> reciprocal -> broadcast multiply
                nc.vector.tensor_copy(out=dn[0:1, :], in_=py_a[HD:HD + 1, :])
                nc.vector.tensor_copy(out=dn[32:33, :], in_=py_b[HD:HD + 1, :])
                rc32 = sm.tile([33, TQ], F32, tag="rc32")
                nc.vector.reciprocal_approx_fast(out=rc32[:, :], in_=dn[:, :])
                rc = sm.tile([33, TQ], BF16, tag="rc")
                nc.vector.tensor_copy(out=rc[:, :], in_=rc32[:, :])
                pb = ps.tile([128, TQ], F32, tag="b", name=f"pb_{tq}_{pr}")
                for i in range(2):
                    nc.tensor.matmul(
                        pb[64 * i:64 * i + 64, :], onesr[32 * i:32 * i + 1, :],
                        rc[32 * i:32 * i + 1, :],
                        start=True, stop=True)
                bc = sm.tile([128, TQ], F32, tag="bc")
                nc.vector.tensor_copy(out=bc[:], in_=pb[:])
                for (i, pyt) in ((0, py_a), (1, py_b)):
                    row0 = 64 * i
                    nc.vector.tensor_mul(
                        out=yT[pr][row0:row0 + 64, tq * TQ:(tq + 1) * TQ],
                        in0=pyt[0:HD, :], in1=bc[64 * i:64 * i + 64, :])

            # ---- main schedule
            for tq in range(NTQ):
                project_qk(tq)
                for t in range(4 * tq, 4 * tq + 4):
                    project_v(t)
                attention(tq, 0)
                attention(tq, 1)
                oproj_queue.extend((tq, e8) for e8 in range(8))
            while oproj_queue:
                oproj_chain(*oproj_queue.pop(0))

    nc.compile()
    return nc


def _get_nc():
    if 'nc' not in _CACHE:
        _CACHE['nc'] = _build()
    return _CACHE['nc']


def _make_in_maps(x, Wq, bq, Wk, bk, Wv, bv, Wo, bo):
    import ml_dtypes
    E4 = ml_dtypes.float8_e4m3
    BF = ml_dtypes.bfloat16

    x = np.asarray(x, dtype=np.float32)
    Wq = np.asarray(Wq, dtype=np.float32)
    Wk = np.asarray(Wk, dtype=np.float32)
    Wv = np.asarray(Wv, dtype=np.float32)
    Wo = np.asarray(Wo, dtype=np.float32)
    bq = np.asarray(bq, dtype=np.float32)
    bk = np.asarray(bk, dtype=np.float32)
    bv = np.asarray(bv, dtype=np.float32)

    onesr = np.ones((33, HD), dtype=BF)

    def wbf_layout(W, hs):
        wt = W[hs].T.astype(BF)                       # [1024, 256]
        return np.ascontiguousarray(
            wt.reshape(DM, 128, E).transpose(1, 0, 2))
    def wf8_layout(W, hs):
        wt = (W[hs].T * WS).astype(E4)
        return np.ascontiguousarray(
            wt.reshape(DM // 2, 2, 128, E).transpose(2, 0, 1, 3))

    in_maps = []
    for c in range(NCORES):
        b, g = divmod(c, 4)
        hs = slice(g * E, (g + 1) * E)
        xT = np.ascontiguousarray(x[b].T)             # [1024, 2048]
        x4 = xT.reshape(DM, 128, T).transpose(1, 0, 2)  # [128, 8, 2048]
        in_maps.append({
            "xbf": np.ascontiguousarray(x4[:, :, 0:TQ]).astype(BF),
            "xf8": np.ascontiguousarray(x4[:, :, TQ:]).astype(E4),
            "wqbf": wbf_layout(Wq, hs), "wqf8": wf8_layout(Wq, hs),
            "wkbf": wbf_layout(Wk, hs), "wkf8": wf8_layout(Wk, hs),
            "wvbf": wbf_layout(Wv, hs), "wvf8": wf8_layout(Wv, hs),
            "wo": np.ascontiguousarray(Wo[:, hs].T).astype(BF),
            "bq": np.ascontiguousarray(bq[hs].reshape(E, 1)),
            "bk": np.ascontiguousarray(bk[hs].reshape(E, 1)),
            "bvbf": bv[hs].reshape(1, E).astype(BF),
            "bvf8": (bv[hs].reshape(1, E) * WS).astype(E4),
            "onebf": np.ones((1, 128), dtype=BF),
            "onef8": np.ones((1, 128), dtype=E4),
            "onesr": onesr,
        })
    return in_maps


def kernel(x, Wq, bq, Wk, bk, Wv, bv, Wo, bo, _run_kwargs=None):
    nc = _get_nc()
    in_maps = _make_in_maps(x, Wq, bq, Wk, bk, Wv, bv, Wo, bo)
    last_err = None
    for _attempt in range(3):
        try:
            res = run_bass_kernel_spmd(nc, in_maps, core_ids=list(range(NCORES)),
                                       **(_run_kwargs or {}))
            break
        except Exception as e:  # transient NRT/device hiccups: retry
            last_err = e
            import time as _time
            _time.sleep(2.0)
    else:
        raise last_err
    bo = np.asarray(bo, dtype=np.float32)
    out = np.empty((B, T, D), dtype=np.float32)
    for b in range(B):
        acc = res.results[4 * b]["outT"].astype(np.float32)
        for g in range(1, 4):
            acc += res.results[4 * b + g]["outT"].astype(np.float32)
        out[b] = acc.T + bo
    if _run_kwargs:
        _CACHE['last_results'] = res
    return out


# revision 26
# speedup vs baseline: 1.2092x; 1.0452x over previous
"""Causal self-attention (B=2, T=2048, D=1024, H=16) on 8 TRN2 NeuronCores.

Sharding: data-parallel over batch (cores 0-3 -> batch 0, cores 4-7 -> batch 1),
tensor-parallel over heads (4 heads / 256 output dims per core). Each core
computes q/k/v projections for its heads, causal flash-style attention, and a
partial output projection (contraction over its 256 dims of Wo). The host sums
the 4 partials per batch and adds bo.

Precision: fp8 noise on attention inputs is only harmful for EARLY tokens
(queries that average few keys get no noise suppression).  So queries/tokens
0..511 (tq=0) run fully in bf16, while tq>=1 uses fp8: projections as
fp8e4m3 DoubleRow matmuls (2 contraction tiles per instruction, 0.5 cyc/row),
S matmuls in fp8e3m4, attention*V in fp8e4m3 DoubleRow over key-block pairs.
fp8 weights are pre-scaled by 32 on the host (removed in the psum->sbuf
casts); exp() runs with bias=-2 so attention weights stay well inside e4m3
range (the softmax normalization cancels the constant).  The output
projection runs in bf16.  Validated full-output rel err ~6e-3.
"""
import sys

sys.path.insert(0, '/opt/trn_rl_repo')

import numpy as np

import concourse.bass as bass  # noqa: F401  (import keeps bass registered)
import concourse.mybir as mybir
import concourse.tile as tile
from concourse import bacc
from concourse.bass_utils import run_bass_kernel_spmd

F32 = mybir.dt.float32
F32R = mybir.dt.float32r
BF16 = mybir.dt.bfloat16
F8E4 = mybir.dt.float8e4   # e4m3
F8E3 = mybir.dt.float8e3   # e3m4
AF = mybir.ActivationFunctionType
ALU = mybir.AluOpType
DRM = mybir.MatmulPerfMode.DoubleRow

B, T, D, H, HD = 2, 2048, 1024, 16, 64
NCORES = 8
E = 256          # output dims per core (4 heads x 64)
DM = 8           # d_model chunks of 128
TQ = 512
NTQ = T // TQ    # 4
TL = T - TQ      # late tokens (fp8 region)
WS = 32.0        # host-side fp8 weight pre-scale (removed in casts)

_CACHE = {}


def _build():
    nc = bacc.Bacc("TRN2", target_bir_lowering=False, debug=False)

    xbf_d = nc.dram_tensor("xbf", [128, DM, TQ], BF16, kind="ExternalInput")
    xf8_d = nc.dram_tensor("xf8", [128, DM, TL], F8E4, kind="ExternalInput")
    wbf_d = {w: nc.dram_tensor(f"w{w}bf", [128, DM, E], BF16, kind="ExternalInput")
             for w in "qkv"}
    wf8_d = {w: nc.dram_tensor(f"w{w}f8", [128, DM // 2, 2, E], F8E4,
                               kind="ExternalInput")
             for w in "qkv"}
    wo_d = nc.dram_tensor("wo", [E, D], BF16, kind="ExternalInput")
    cf32_d = nc.dram_tensor("cf32", [128, 4], F32, kind="ExternalInput")
    cbf_d = nc.dram_tensor("cbf", [33, 128 + E], BF16, kind="ExternalInput")
    cf8_d = nc.dram_tensor("cf8", [1, 128 + E], F8E4, kind="ExternalInput")
    outT = nc.dram_tensor("outT", [D, T], BF16, kind="ExternalOutput")

    with tile.TileContext(nc) as tc, nc.allow_low_precision(reason="fp8/bf16 attn"):
        with (
            tc.tile_pool(name="persist", bufs=1) as pp,
            tc.tile_pool(name="es", bufs=3) as esp,
            tc.tile_pool(name="esd", bufs=4) as esd,
            tc.tile_pool(name="ostage", bufs=3) as op_pool,
            tc.tile_pool(name="small", bufs=1) as sm,
            tc.tile_pool(name="psum", bufs=2, space="PSUM") as ps,
        ):
            # ---- persistent SBUF tiles
            xbf = pp.tile([128, DM, TQ], BF16, tag="xbf", name="xbf")
            xf8 = pp.tile([128, DM, TL], F8E4, tag="xf8", name="xf8")
            wbf = {w: pp.tile([128, DM, E], BF16, tag=f"w{w}bf", name=f"w{w}bf")
                   for w in "qkv"}
            wf8 = {w: pp.tile([128, DM // 2, 2, E], F8E4, tag=f"w{w}f8",
                               name=f"w{w}f8")
                   for w in "qkv"}
            wo_sb = [pp.tile([128, D], BF16, tag=f"wo{d2}", name=f"wo{d2}")
                     for d2 in range(2)]
            cf32 = pp.tile([128, 4], F32, tag="cf32")
            cbf = pp.tile([33, 128 + E], BF16, tag="cbf")
            cf8 = pp.tile([1, 128 + E], F8E4, tag="cf8")
            bq_sb = [cf32[:, 0:1], cf32[:, 1:2]]
            bk_sb = [cf32[:, 2:3], cf32[:, 3:4]]
            onebf = cbf[0:1, 0:128]
            bvbf = cbf[0:1, 128:128 + E]
            onef8 = cf8[0:1, 0:128]
            bvf8 = cf8[0:1, 128:128 + E]
            nbias = pp.tile([128, 1], F32, tag="nbias")
            nc.vector.memset(nbias[:], -2.0)
            dn = pp.tile([33, TQ], F32, tag="dn")
            nc.vector.memset(dn[:], 1.0)

            # dual-queue startup: sync carries the first chain's deps
            # (wq chunk c, x chunk c) interleaved; the activation queue
            # brings in everything else in parallel.
            for c in range(DM):
                nc.sync.dma_start(out=wbf["q"][:, c, :], in_=wbf_d["q"][:, c, :])
                nc.sync.dma_start(out=xbf[:, c, :], in_=xbf_d[:, c, :])
            nc.scalar.dma_start(out=wbf["k"][:], in_=wbf_d["k"][:, :, :])
            nc.scalar.dma_start(out=cf32[:], in_=cf32_d[:, :])
            nc.scalar.dma_start(out=wbf["v"][:], in_=wbf_d["v"][:, :, :])
            nc.scalar.dma_start(out=cbf[:], in_=cbf_d[:, :])
            nc.scalar.dma_start(out=cf8[:], in_=cf8_d[:, :])
            for w in "qkv":
                nc.scalar.dma_start(out=wf8[w][:], in_=wf8_d[w][:, :, :, :])
            for d2 in range(2):
                nc.scalar.dma_start(out=wo_sb[d2][:],
                                    in_=wo_d[d2 * 128:(d2 + 1) * 128, :])
            for tq in range(1, NTQ):
                lo = (tq - 1) * TQ
                for c in range(DM):
                    nc.sync.dma_start(out=xf8[:, c, lo:lo + TQ],
                                      in_=xf8_d[:, c, lo:lo + TQ])

            # q/k: bf16 tiles cover tokens 0..511, fp8e3 tiles cover all tokens
            # (early-key slices are needed by late queries).
            qT_bf = [pp.tile([128, TQ], BF16, tag=f"qbf{i}", name=f"qbf{i}")
                     for i in range(2)]
            kT_bf = [pp.tile([128, TQ], BF16, tag=f"kbf{i}", name=f"kbf{i}")
                     for i in range(2)]
            qT_f8 = [pp.tile([128, T], F8E3, tag=f"qf8{i}", name=f"qf8{i}")
                     for i in range(2)]
            kT_f8 = [pp.tile([128, T], F8E3, tag=f"kf8{i}", name=f"kf8{i}")
                     for i in range(2)]
            # v pair tiles: [keys, pair-member, head, HD+ones-col] (all blocks)
            v_sb = [pp.tile([128, 2, 4, 68], F8E4, tag=f"v{i}", name=f"v{i}")
                    for i in range(8)]
            for i in range(8):
                nc.vector.memset(v_sb[i][:, :, :, HD:HD + 1], 1.0)
            # bf16 v for key blocks 0..3 (early queries)
            v_bf = [pp.tile([128, 4, HD + 1], BF16, tag=f"vb{i}", name=f"vb{i}")
                    for i in range(4)]
            for i in range(4):
                nc.vector.memset(v_bf[i][:, :, HD:HD + 1], 1.0)
            yT = [pp.tile([128, T], BF16, tag=f"yT{i}", name=f"yT{i}") for i in range(2)]

            def project_qk(tq):
                for (w, b_sb, dst_bf, dst_f8) in (
                        ("q", bq_sb, qT_bf, qT_f8), ("k", bk_sb, kT_bf, kT_f8)):
                    for e2 in range(2):
                        pt = ps.tile([128, 1024], F32, tag="S",
                                     name=f"pqk_{w}_{tq}_{e2}")
                        if tq == 0:
                            for c in range(DM):
                                nc.tensor.matmul(
                                    pt[:, 0:TQ],
                                    wbf[w][:, c, e2 * 128:(e2 + 1) * 128],
                                    xbf[:, c, :],
                                    start=(c == 0), stop=(c == DM - 1))
                            nc.vector.tensor_scalar_add(
                                out=dst_bf[e2][:, :],
                                in0=pt[:, 0:TQ], scalar1=b_sb[e2])
                            if w == "k":
                                nc.gpsimd.tensor_copy(
                                    out=dst_f8[e2][:, 0:TQ],
                                    in_=dst_bf[e2][:, :])
                        else:
                            lo = (tq - 1) * TQ
                            for c in range(DM // 2):
                                nc.tensor.matmul(
                                    pt[:, 0:TQ],
                                    wf8[w][:, c, :, e2 * 128:(e2 + 1) * 128],
                                    xf8[:, 2 * c:2 * c + 2, lo:lo + TQ],
                                    start=(c == 0), stop=(c == DM // 2 - 1),
                                    perf_mode=DRM)
                            nc.vector.tensor_scalar(
                                out=dst_f8[e2][:, tq * TQ:(tq + 1) * TQ],
                                in0=pt[:, 0:TQ],
                                scalar1=1.0 / WS, scalar2=b_sb[e2],
                                op0=ALU.mult, op1=ALU.add)

            def project_v(t):
                pv = ps.tile([128, E], F32, tag="y", name=f"pv_{t}")
                if t < 4:
                    for c in range(DM):
                        nc.tensor.matmul(
                            pv[:],
                            xbf[:, c, t * 128:(t + 1) * 128],
                            wbf["v"][:, c, :],
                            start=(c == 0), stop=False)
                    nc.tensor.matmul(
                        pv[:], onebf, bvbf, start=False, stop=True)
                    nc.vector.tensor_copy(
                        out=v_bf[t][:, :, 0:HD],
                        in_=pv[:].rearrange("p (h d) -> p h d", h=4))
                    nc.gpsimd.tensor_copy(
                        out=v_sb[t // 2][:, t % 2, :, 0:HD],
                        in_=v_bf[t][:, :, 0:HD])
                else:
                    lo = t * 128 - TQ
                    for c in range(DM // 2):
                        nc.tensor.matmul(
                            pv[:],
                            xf8[:, 2 * c:2 * c + 2, lo:lo + 128],
                            wf8["v"][:, c, :, :],
                            start=(c == 0), stop=False, perf_mode=DRM)
                    nc.tensor.matmul(
                        pv[:], onef8, bvf8, start=False, stop=True)
                    nc.vector.tensor_scalar_mul(
                        out=v_sb[t // 2][:, t % 2, :, 0:HD],
                        in0=pv[:].rearrange("p (h d) -> p h d", h=4),
                        scalar1=1.0 / WS)

            oproj_queue = []

            def oproj_chain(tq_o, e8):
                pt = ps.tile([128, TQ], F32, tag="b", name=f"poc_{tq_o}_{e8}")
                for d2 in range(2):
                    nc.tensor.matmul(
                        pt[:, 0:TQ],
                        wo_sb[d2][:, e8 * 128:(e8 + 1) * 128],
                        yT[d2][:, tq_o * TQ:(tq_o + 1) * TQ],
                        start=(d2 == 0), stop=(d2 == 1))
                ot = op_pool.tile([128, TQ], BF16, tag="ostage", name=f"oto_{tq_o}_{e8}")
                if e8 % 4 == 0:
                    nc.scalar.copy(out=ot[:], in_=pt[:, 0:TQ])
                else:
                    nc.vector.tensor_copy(out=ot[:], in_=pt[:, 0:TQ])
                nc.sync.dma_start(
                    out=outT[e8 * 128:(e8 + 1) * 128, tq_o * TQ:(tq_o + 1) * TQ],
                    in_=ot[:])

            def pop_filler():
                if oproj_queue:
                    oproj_chain(*oproj_queue.pop(0))

            def attention(tq, pr):
                bf = (tq == 0)
                kt = kT_bf[pr] if bf else kT_f8[pr]
                qt = qT_bf[pr] if bf else qT_f8[pr]
                qof = 0 if bf else tq * TQ
                py_a = ps.tile([HD + 1, TQ], F32, tag="y", name=f"pya_{tq}_{pr}")
                py_b = ps.tile([HD + 1, TQ], F32, tag="y", name=f"pyb_{tq}_{pr}")
                py = (py_a, py_b)
                npair = 0 if bf else 2 * tq
                units = [("pair", i) for i in range(npair)] + \
                        [("diag", o) for o in range(4)]

                def s_unit(u):
                    kind, idx = u
                    if kind == "pair":
                        est = esp.tile([128, 2, 2, TQ], F8E4, tag="es",
                                       name=f"es_{tq}_{pr}_{idx}")
                        for j in range(2):
                            tk = 2 * idx + j
                            ps_s = ps.tile([128, 1024], F32, tag="S",
                                           name=f"ps_{tq}_{pr}_{tk}")
                            for h in range(2):
                                nc.tensor.matmul(
                                    ps_s[:, h * TQ:(h + 1) * TQ],
                                    kt[64 * h:64 * h + 64, tk * 128:(tk + 1) * 128],
                                    qt[64 * h:64 * h + 64, qof:qof + TQ],
                                    start=True, stop=True)
                            nc.scalar.activation(
                                est[:, j, :, :], ps_s[:], AF.Exp,
                                bias=nbias[:], scale=0.125)
                        return (est, 0)
                    # diagonal block: only columns >= c0 are live
                    o = idx
                    tk = 4 * tq + o
                    c0 = 128 * o
                    n = TQ - c0
                    edt = BF16 if bf else F8E4
                    ps_s = ps.tile([128, 1024], F32, tag="S",
                                   name=f"psd_{tq}_{pr}_{o}")
                    ps2 = ps_s[:].rearrange("p (h q) -> p h q", h=2)
                    for h in range(2):
                        nc.tensor.matmul(
                            ps_s[:, h * TQ + c0:(h + 1) * TQ],
                            kt[64 * h:64 * h + 64, tk * 128:(tk + 1) * 128],
                            qt[64 * h:64 * h + 64, qof + c0:qof + TQ],
                            start=True, stop=True)
                    es_t = esd.tile([128, 2, TQ], edt, tag="esd",
                                    name=f"esd_{tq}_{pr}_{o}")
                    nc.scalar.activation(
                        es_t[:, :, c0:TQ], ps2[:, :, c0:TQ], AF.Exp,
                        bias=nbias[:], scale=0.125)
                    nc.gpsimd.affine_select(
                        out=es_t[:, :, c0:c0 + 128],
                        in_=es_t[:, :, c0:c0 + 128],
                        compare_op=ALU.is_ge,
                        fill=0.0,
                        base=0,
                        pattern=[[0, 2], [1, 128]],
                        channel_multiplier=-1)
                    return (es_t, c0)

                def y_unit(u, es, c0):
                    kind, idx = u
                    if kind == "pair":
                        for h in range(2):
                            nc.tensor.matmul(
                                py[h][:, :],
                                v_sb[idx][:, :, 2 * pr + h, 0:HD + 1],
                                es[:, :, h, :],
                                start=(idx == 0), stop=False, perf_mode=DRM)
                    else:
                        tk = 4 * tq + idx
                        for h in range(2):
                            vt = (v_bf[tk][:, 2 * pr + h, :] if bf
                                  else v_sb[tk // 2][:, tk % 2, 2 * pr + h, 0:HD + 1])
                            nc.tensor.matmul(
                                py[h][:, c0:TQ],
                                vt,
                                es[:, h, c0:TQ],
                                start=(npair == 0 and idx == 0), stop=(idx == 3))

                prev = None
                for u in units:
                    cur = (u, s_unit(u))
                    if prev is not None:
                        y_unit(prev[0], *prev[1])
                    pop_filler()
                    prev = cur
                y_unit(prev[0], *prev[1])

                # softmax denominators -> reciprocal -> broadcast multiply
                nc.vector.tensor_copy(out=dn[0:1, :], in_=py_a[HD:HD + 1, :])
                nc.vector.tensor_copy(out=dn[32:33, :], in_=py_b[HD:HD + 1, :])
                rc32 = sm.tile([33, TQ], F32, tag="rc32")
                nc.vector.reciprocal_approx_fast(out=rc32[:, :], in_=dn[:, :])
                rc = sm.tile([33, TQ], BF16, tag="rc")
                nc.vector.tensor_copy(out=rc[:, :], in_=rc32[:, :])
                pb = ps.tile([128, TQ], F32, tag="b", name=f"pb_{tq}_{pr}")
                for i in range(2):
                    nc.tensor.matmul(
                        pb[64 * i:64 * i + 64, :], cbf[32 * i:32 * i + 1, 0:HD],
                        rc[32 * i:32 * i + 1, :],
                        start=True, stop=True)
                bc = sm.tile([128, TQ], F32, tag="bc")
                nc.vector.tensor_copy(out=bc[:], in_=pb[:])
                for (i, pyt) in ((0, py_a), (1, py_b)):
                    row0 = 64 * i
                    nc.vector.tensor_mul(
                        out=yT[pr][row0:row0 + 64, tq * TQ:(tq + 1) * TQ],
                        in0=pyt[0:HD, :], in1=bc[64 * i:64 * i + 64, :])

            # ---- main schedule
            for tq in range(NTQ):
                project_qk(tq)
                for t in range(4 * tq, 4 * tq + 4):
                    project_v(t)
                attention(tq, 0)
                attention(tq, 1)
                oproj_queue.extend((tq, e8) for e8 in range(8))
            while oproj_queue:
                oproj_chain(*oproj_queue.pop(0))

    nc.compile()
    return nc


def _get_nc():
    if 'nc' not in _CACHE:
        _CACHE['nc'] = _build()
    return _CACHE['nc']


def _make_in_maps(x, Wq, bq, Wk, bk, Wv, bv, Wo, bo):
    import ml_dtypes
    E4 = ml_dtypes.float8_e4m3
    BF = ml_dtypes.bfloat16

    x = np.asarray(x, dtype=np.float32)
    Wq = np.asarray(Wq, dtype=np.float32)
    Wk = np.asarray(Wk, dtype=np.float32)
    Wv = np.asarray(Wv, dtype=np.float32)
    Wo = np.asarray(Wo, dtype=np.float32)
    bq = np.asarray(bq, dtype=np.float32)
    bk = np.asarray(bk, dtype=np.float32)
    bv = np.asarray(bv, dtype=np.float32)



    def wbf_layout(W, hs):
        wt = W[hs].T.astype(BF)                       # [1024, 256]
        return np.ascontiguousarray(
            wt.reshape(DM, 128, E).transpose(1, 0, 2))
    def wf8_layout(W, hs):
        wt = (W[hs].T * WS).astype(E4)
        return np.ascontiguousarray(
            wt.reshape(DM // 2, 2, 128, E).transpose(2, 0, 1, 3))

    in_maps = []
    for c in range(NCORES):
        b, g = divmod(c, 4)
        hs = slice(g * E, (g + 1) * E)
        xT = np.ascontiguousarray(x[b].T)             # [1024, 2048]
        x4 = xT.reshape(DM, 128, T).transpose(1, 0, 2)  # [128, 8, 2048]
        cf32 = np.stack([bq[hs][0:128], bq[hs][128:256],
                         bk[hs][0:128], bk[hs][128:256]], axis=1)
        cbf = np.zeros((33, 128 + E), dtype=BF)
        cbf[:, 0:128] = 1.0
        cbf[0, 128:] = bv[hs].astype(BF)
        cf8 = np.zeros((1, 128 + E), dtype=E4)
        cf8[:, 0:128] = 1.0
        cf8[0, 128:] = (bv[hs] * WS).astype(E4)
        in_maps.append({
            "xbf": np.ascontiguousarray(x4[:, :, 0:TQ]).astype(BF),
            "xf8": np.ascontiguousarray(x4[:, :, TQ:]).astype(E4),
            "wqbf": wbf_layout(Wq, hs), "wqf8": wf8_layout(Wq, hs),
            "wkbf": wbf_layout(Wk, hs), "wkf8": wf8_layout(Wk, hs),
            "wvbf": wbf_layout(Wv, hs), "wvf8": wf8_layout(Wv, hs),
            "wo": np.ascontiguousarray(Wo[:, hs].T).astype(BF),
            "cf32": np.ascontiguousarray(cf32, dtype=np.float32),
            "cbf": cbf, "cf8": cf8,
        })
    return in_maps


def kernel(x, Wq, bq, Wk, bk, Wv, bv, Wo, bo, _run_kwargs=None):
    nc = _get_nc()
    in_maps = _make_in_maps(x, Wq, bq, Wk, bk, Wv, bv, Wo, bo)
    last_err = None
    for _attempt in range(3):
        try:
            res = run_bass_kernel_spmd(nc, in_maps, core_ids=list(range(NCORES)),
                                       **(_run_kwargs or {}))
            break
        except Exception as e:  # transient NRT/device hiccups: retry
            last_err = e
            import time as _time
            _time.sleep(2.0)
    else:
        raise last_err
    bo = np.asarray(bo, dtype=np.float32)
    out = np.empty((B, T, D), dtype=np.float32)
    for b in range(B):
        acc = res.results[4 * b]["outT"].astype(np.float32)
        for g in range(1, 4):
            acc += res.results[4 * b + g]["outT"].astype(np.float32)
        out[b] = acc.T + bo
    if _run_kwargs:
        _CACHE['last_results'] = res
    return out


# revision 28
# speedup vs baseline: 1.2564x; 1.0391x over previous
"""Causal self-attention (B=2, T=2048, D=1024, H=16) on 8 TRN2 NeuronCores.

Sharding: data-parallel over batch (cores 0-3 -> batch 0, cores 4-7 -> batch 1),
tensor-parallel over heads (4 heads / 256 output dims per core). Each core
computes q/k/v projections for its heads, causal flash-style attention, and a
partial output projection (contraction over its 256 dims of Wo). The host sums
the 4 partials per batch and adds bo.

Precision: fp8 noise on attention inputs is only harmful for EARLY tokens
(queries that average few keys get no noise suppression).  So queries/tokens
0..511 (tq=0) run fully in bf16, while tq>=1 uses fp8: projections as
fp8e4m3 DoubleRow matmuls (2 contraction tiles per instruction, 0.5 cyc/row),
S matmuls in fp8e3m4, attention*V in fp8e4m3 DoubleRow over key-block pairs.
fp8 weights are pre-scaled by 32 on the host (removed in the psum->sbuf
casts); exp() runs with bias=-2 so attention weights stay well inside e4m3
range (the softmax normalization cancels the constant).  The output
projection runs in bf16.  Validated full-output rel err ~6e-3.
"""
import sys

sys.path.insert(0, '/opt/trn_rl_repo')

import numpy as np

import concourse.bass as bass  # noqa: F401  (import keeps bass registered)
import concourse.mybir as mybir
import concourse.tile as tile
from concourse import bacc
from concourse.bass_utils import run_bass_kernel_spmd

F32 = mybir.dt.float32
F32R = mybir.dt.float32r
BF16 = mybir.dt.bfloat16
F8E4 = mybir.dt.float8e4   # e4m3
F8E3 = mybir.dt.float8e3   # e3m4
AF = mybir.ActivationFunctionType
ALU = mybir.AluOpType
DRM = mybir.MatmulPerfMode.DoubleRow

B, T, D, H, HD = 2, 2048, 1024, 16, 64
NCORES = 8
E = 256          # output dims per core (4 heads x 64)
DM = 8           # d_model chunks of 128
TQ = 512
NTQ = T // TQ    # 4
TL = T - TQ      # late tokens (fp8 region)
WS = 32.0        # host-side fp8 weight pre-scale (removed in casts)

_CACHE = {}


def _build():
    nc = bacc.Bacc("TRN2", target_bir_lowering=False, debug=False)

    xbf_d = nc.dram_tensor("xbf", [128, DM, TQ], BF16, kind="ExternalInput")
    xf8_d = nc.dram_tensor("xf8", [128, DM, TL], F8E4, kind="ExternalInput")
    wbf_d = {w: nc.dram_tensor(f"w{w}bf", [128, DM, E], BF16, kind="ExternalInput")
             for w in "qkv"}
    wf8_d = {w: nc.dram_tensor(f"w{w}f8", [128, DM // 2, 2, E], F8E4,
                               kind="ExternalInput")
             for w in "qkv"}
    wo_d = nc.dram_tensor("wo", [E, D], BF16, kind="ExternalInput")
    cf32_d = nc.dram_tensor("cf32", [128, 4], F32, kind="ExternalInput")
    cbf_d = nc.dram_tensor("cbf", [33, 128 + E], BF16, kind="ExternalInput")
    cf8_d = nc.dram_tensor("cf8", [1, 128 + E], F8E4, kind="ExternalInput")
    outT = nc.dram_tensor("outT", [D, T], BF16, kind="ExternalOutput")

    with tile.TileContext(nc) as tc, nc.allow_low_precision(reason="fp8/bf16 attn"):
        with (
            tc.tile_pool(name="persist", bufs=1) as pp,
            tc.tile_pool(name="es", bufs=3) as esp,
            tc.tile_pool(name="esd", bufs=4) as esd,
            tc.tile_pool(name="ostage", bufs=3) as op_pool,
            tc.tile_pool(name="small", bufs=1) as sm,
            tc.tile_pool(name="psum", bufs=2, space="PSUM") as ps,
        ):
            # ---- persistent SBUF tiles
            xbf = pp.tile([128, DM, TQ], BF16, tag="xbf", name="xbf")
            xf8 = pp.tile([128, DM, TL], F8E4, tag="xf8", name="xf8")
            wbf = {w: pp.tile([128, DM, E], BF16, tag=f"w{w}bf", name=f"w{w}bf")
                   for w in "qkv"}
            wf8 = {w: pp.tile([128, DM // 2, 2, E], F8E4, tag=f"w{w}f8",
                               name=f"w{w}f8")
                   for w in "qkv"}
            wo_sb = [pp.tile([128, D], BF16, tag=f"wo{d2}", name=f"wo{d2}")
                     for d2 in range(2)]
            cf32 = pp.tile([128, 4], F32, tag="cf32")
            cbf = pp.tile([33, 128 + E], BF16, tag="cbf")
            cf8 = pp.tile([1, 128 + E], F8E4, tag="cf8")
            bq_sb = [cf32[:, 0:1], cf32[:, 1:2]]
            bk_sb = [cf32[:, 2:3], cf32[:, 3:4]]
            onebf = cbf[0:1, 0:128]
            bvbf = cbf[0:1, 128:128 + E]
            onef8 = cf8[0:1, 0:128]
            bvf8 = cf8[0:1, 128:128 + E]
            nbias = pp.tile([128, 1], F32, tag="nbias")
            nc.vector.memset(nbias[:], -2.0)
            dn = pp.tile([33, TQ], F32, tag="dn")
            nc.vector.memset(dn[:], 1.0)

            # dual-queue startup: sync carries the first chain's deps
            # (wq chunk c, x chunk c) interleaved; the activation queue
            # brings in everything else in parallel.
            for c in range(DM):
                nc.sync.dma_start(out=wbf["q"][:, c, :], in_=wbf_d["q"][:, c, :])
                nc.sync.dma_start(out=xbf[:, c, :], in_=xbf_d[:, c, :])
            nc.scalar.dma_start(out=wbf["k"][:], in_=wbf_d["k"][:, :, :])
            nc.scalar.dma_start(out=cf32[:], in_=cf32_d[:, :])
            nc.scalar.dma_start(out=wbf["v"][:], in_=wbf_d["v"][:, :, :])
            nc.scalar.dma_start(out=cbf[:], in_=cbf_d[:, :])
            nc.scalar.dma_start(out=cf8[:], in_=cf8_d[:, :])
            for w in "qkv":
                nc.scalar.dma_start(out=wf8[w][:], in_=wf8_d[w][:, :, :, :])
            for d2 in range(2):
                nc.scalar.dma_start(out=wo_sb[d2][:],
                                    in_=wo_d[d2 * 128:(d2 + 1) * 128, :])
            for tq in range(1, NTQ):
                lo = (tq - 1) * TQ
                for c in range(DM):
                    nc.sync.dma_start(out=xf8[:, c, lo:lo + TQ],
                                      in_=xf8_d[:, c, lo:lo + TQ])

            # q/k: bf16 tiles cover tokens 0..511, fp8e3 tiles cover all tokens
            # (early-key slices are needed by late queries).
            qT_bf = [pp.tile([128, TQ], BF16, tag=f"qbf{i}", name=f"qbf{i}")
                     for i in range(2)]
            kT_bf = [pp.tile([128, TQ], BF16, tag=f"kbf{i}", name=f"kbf{i}")
                     for i in range(2)]
            qT_f8 = [pp.tile([128, T], F8E3, tag=f"qf8{i}", name=f"qf8{i}")
                     for i in range(2)]
            kT_f8 = [pp.tile([128, T], F8E3, tag=f"kf8{i}", name=f"kf8{i}")
                     for i in range(2)]
            # v pair tiles: [keys, pair-member, head, HD+ones-col] (all blocks)
            v_sb = [pp.tile([128, 2, 4, 68], F8E4, tag=f"v{i}", name=f"v{i}")
                    for i in range(8)]
            for i in range(8):
                nc.vector.memset(v_sb[i][:, :, :, HD:HD + 1], 1.0)
            # bf16 v for key blocks 0..3 (early queries)
            v_bf = [pp.tile([128, 4, HD + 1], BF16, tag=f"vb{i}", name=f"vb{i}")
                    for i in range(4)]
            for i in range(4):
                nc.vector.memset(v_bf[i][:, :, HD:HD + 1], 1.0)
            yT = [pp.tile([128, T], BF16, tag=f"yT{i}", name=f"yT{i}") for i in range(2)]

            QKW = (("q", bq_sb, qT_bf, qT_f8), ("k", bk_sb, kT_bf, kT_f8))

            def qk_chain(tq, wi, e2):
                w, b_sb, dst_bf, dst_f8 = QKW[wi]
                pt = ps.tile([128, 1024], F32, tag="S",
                             name=f"pqk_{w}_{tq}_{e2}")
                if tq == 0:
                    for c in range(DM):
                        nc.tensor.matmul(
                            pt[:, 0:TQ],
                            wbf[w][:, c, e2 * 128:(e2 + 1) * 128],
                            xbf[:, c, :],
                            start=(c == 0), stop=(c == DM - 1))
                    nc.vector.tensor_scalar_add(
                        out=dst_bf[e2][:, :],
                        in0=pt[:, 0:TQ], scalar1=b_sb[e2])
                    if w == "k":
                        nc.gpsimd.tensor_copy(
                            out=dst_f8[e2][:, 0:TQ],
                            in_=dst_bf[e2][:, :])
                else:
                    lo = (tq - 1) * TQ
                    for c in range(DM // 2):
                        nc.tensor.matmul(
                            pt[:, 0:TQ],
                            wf8[w][:, c, :, e2 * 128:(e2 + 1) * 128],
                            xf8[:, 2 * c:2 * c + 2, lo:lo + TQ],
                            start=(c == 0), stop=(c == DM // 2 - 1),
                            perf_mode=DRM)
                    nc.vector.tensor_scalar(
                        out=dst_f8[e2][:, tq * TQ:(tq + 1) * TQ],
                        in0=pt[:, 0:TQ],
                        scalar1=1.0 / WS, scalar2=b_sb[e2],
                        op0=ALU.mult, op1=ALU.add)

            def project_qk(tq):
                for wi in range(2):
                    for e2 in range(2):
                        qk_chain(tq, wi, e2)

            def project_v(t):
                pv = ps.tile([128, E], F32, tag="b", name=f"pv_{t}")
                if t < 4:
                    for c in range(DM):
                        nc.tensor.matmul(
                            pv[:],
                            xbf[:, c, t * 128:(t + 1) * 128],
                            wbf["v"][:, c, :],
                            start=(c == 0), stop=False)
                    nc.tensor.matmul(
                        pv[:], onebf, bvbf, start=False, stop=True)
                    nc.vector.tensor_copy(
                        out=v_bf[t][:, :, 0:HD],
                        in_=pv[:].rearrange("p (h d) -> p h d", h=4))
                    nc.gpsimd.tensor_copy(
                        out=v_sb[t // 2][:, t % 2, :, 0:HD],
                        in_=v_bf[t][:, :, 0:HD])
                else:
                    lo = t * 128 - TQ
                    for c in range(DM // 2):
                        nc.tensor.matmul(
                            pv[:],
                            xf8[:, 2 * c:2 * c + 2, lo:lo + 128],
                            wf8["v"][:, c, :, :],
                            start=(c == 0), stop=False, perf_mode=DRM)
                    nc.tensor.matmul(
                        pv[:], onef8, bvf8, start=False, stop=True)
                    nc.vector.tensor_scalar_mul(
                        out=v_sb[t // 2][:, t % 2, :, 0:HD],
                        in0=pv[:].rearrange("p (h d) -> p h d", h=4),
                        scalar1=1.0 / WS)

            filler_q = []

            def oproj_chain(tq_o, e8):
                pt = ps.tile([128, TQ], F32, tag="b", name=f"poc_{tq_o}_{e8}")
                for d2 in range(2):
                    nc.tensor.matmul(
                        pt[:, 0:TQ],
                        wo_sb[d2][:, e8 * 128:(e8 + 1) * 128],
                        yT[d2][:, tq_o * TQ:(tq_o + 1) * TQ],
                        start=(d2 == 0), stop=(d2 == 1))
                ot = op_pool.tile([128, TQ], BF16, tag="ostage", name=f"oto_{tq_o}_{e8}")
                if e8 % 4 == 0:
                    nc.scalar.copy(out=ot[:], in_=pt[:, 0:TQ])
                else:
                    nc.vector.tensor_copy(out=ot[:], in_=pt[:, 0:TQ])
                nc.sync.dma_start(
                    out=outT[e8 * 128:(e8 + 1) * 128, tq_o * TQ:(tq_o + 1) * TQ],
                    in_=ot[:])

            def pop_filler():
                if filler_q:
                    filler_q.pop(0)()

            def attention(tq, pr):
                bf = (tq == 0)
                kt = kT_bf[pr] if bf else kT_f8[pr]
                qt = qT_bf[pr] if bf else qT_f8[pr]
                qof = 0 if bf else tq * TQ
                py_a = ps.tile([HD + 1, TQ], F32, tag="y", name=f"pya_{tq}_{pr}")
                py_b = ps.tile([HD + 1, TQ], F32, tag="y", name=f"pyb_{tq}_{pr}")
                py = (py_a, py_b)
                npair = 0 if bf else 2 * tq
                units = [("pair", i) for i in range(npair)] + \
                        [("diag", o) for o in range(4)]

                def s_unit(u):
                    kind, idx = u
                    if kind == "pair":
                        est = esp.tile([128, 2, 2, TQ], F8E4, tag="es",
                                       name=f"es_{tq}_{pr}_{idx}")
                        for j in range(2):
                            tk = 2 * idx + j
                            ps_s = ps.tile([128, 1024], F32, tag="S",
                                           name=f"ps_{tq}_{pr}_{tk}")
                            for h in range(2):
                                nc.tensor.matmul(
                                    ps_s[:, h * TQ:(h + 1) * TQ],
                                    kt[64 * h:64 * h + 64, tk * 128:(tk + 1) * 128],
                                    qt[64 * h:64 * h + 64, qof:qof + TQ],
                                    start=True, stop=True)
                            nc.scalar.activation(
                                est[:, j, :, :], ps_s[:], AF.Exp,
                                bias=nbias[:], scale=0.125)
                        return (est, 0)
                    # diagonal block: only columns >= c0 are live
                    o = idx
                    tk = 4 * tq + o
                    c0 = 128 * o
                    n = TQ - c0
                    edt = BF16 if bf else F8E4
                    ps_s = ps.tile([128, 1024], F32, tag="S",
                                   name=f"psd_{tq}_{pr}_{o}")
                    ps2 = ps_s[:].rearrange("p (h q) -> p h q", h=2)
                    for h in range(2):
                        nc.tensor.matmul(
                            ps_s[:, h * TQ + c0:(h + 1) * TQ],
                            kt[64 * h:64 * h + 64, tk * 128:(tk + 1) * 128],
                            qt[64 * h:64 * h + 64, qof + c0:qof + TQ],
                            start=True, stop=True)
                    es_t = esd.tile([128, 2, TQ], edt, tag="esd",
                                    name=f"esd_{tq}_{pr}_{o}")
                    nc.scalar.activation(
                        es_t[:, :, c0:TQ], ps2[:, :, c0:TQ], AF.Exp,
                        bias=nbias[:], scale=0.125)
                    nc.gpsimd.affine_select(
                        out=es_t[:, :, c0:c0 + 128],
                        in_=es_t[:, :, c0:c0 + 128],
                        compare_op=ALU.is_ge,
                        fill=0.0,
                        base=0,
                        pattern=[[0, 2], [1, 128]],
                        channel_multiplier=-1)
                    return (es_t, c0)

                def y_unit(u, es, c0):
                    kind, idx = u
                    if kind == "pair":
                        for h in range(2):
                            nc.tensor.matmul(
                                py[h][:, :],
                                v_sb[idx][:, :, 2 * pr + h, 0:HD + 1],
                                es[:, :, h, :],
                                start=(idx == 0), stop=False, perf_mode=DRM)
                    else:
                        tk = 4 * tq + idx
                        for h in range(2):
                            vt = (v_bf[tk][:, 2 * pr + h, :] if bf
                                  else v_sb[tk // 2][:, tk % 2, 2 * pr + h, 0:HD + 1])
                            nc.tensor.matmul(
                                py[h][:, c0:TQ],
                                vt,
                                es[:, h, c0:TQ],
                                start=(npair == 0 and idx == 0), stop=(idx == 3))

                prev = None
                for u in units:
                    cur = (u, s_unit(u))
                    if prev is not None:
                        y_unit(prev[0], *prev[1])
                    pop_filler()
                    prev = cur
                y_unit(prev[0], *prev[1])

                # softmax denominators -> reciprocal -> broadcast multiply
                nc.vector.tensor_copy(out=dn[0:1, :], in_=py_a[HD:HD + 1, :])
                nc.vector.tensor_copy(out=dn[32:33, :], in_=py_b[HD:HD + 1, :])
                rc32 = sm.tile([33, TQ], F32, tag="rc32")
                nc.vector.reciprocal_approx_fast(out=rc32[:, :], in_=dn[:, :])
                rc = sm.tile([33, TQ], BF16, tag="rc")
                nc.vector.tensor_copy(out=rc[:, :], in_=rc32[:, :])
                pb = ps.tile([128, TQ], F32, tag="b", name=f"pb_{tq}_{pr}")
                for i in range(2):
                    nc.tensor.matmul(
                        pb[64 * i:64 * i + 64, :], cbf[32 * i:32 * i + 1, 0:HD],
                        rc[32 * i:32 * i + 1, :],
                        start=True, stop=True)
                bc = sm.tile([128, TQ], F32, tag="bc")
                nc.vector.tensor_copy(out=bc[:], in_=pb[:])
                for (i, pyt) in ((0, py_a), (1, py_b)):
                    row0 = 64 * i
                    nc.vector.tensor_mul(
                        out=yT[pr][row0:row0 + 64, tq * TQ:(tq + 1) * TQ],
                        in0=pyt[0:HD, :], in1=bc[64 * i:64 * i + 64, :])

            # ---- main schedule: next-tq projections + output projections
            # run as PE fillers inside the attention unit pipeline
            import functools
            project_qk(0)
            for t in range(4):
                project_v(t)
            for tq in range(NTQ):
                if tq + 1 < NTQ:
                    filler_q[:0] = [
                        functools.partial(qk_chain, tq + 1, wi, e2)
                        for wi in range(2) for e2 in range(2)]
                attention(tq, 0)
                if tq + 1 < NTQ:
                    filler_q[:0] = [
                        functools.partial(project_v, t)
                        for t in range(4 * tq + 4, 4 * tq + 8)]
                attention(tq, 1)
                filler_q.extend(
                    functools.partial(oproj_chain, tq, e8) for e8 in range(8))
            while filler_q:
                filler_q.pop(0)()

    nc.compile()
    return nc


def _get_nc():
    if 'nc' not in _CACHE:
        _CACHE['nc'] = _build()
    return _CACHE['nc']


def _make_in_maps(x, Wq, bq, Wk, bk, Wv, bv, Wo, bo):
    import ml_dtypes
    E4 = ml_dtypes.float8_e4m3
    BF = ml_dtypes.bfloat16

    x = np.asarray(x, dtype=np.float32)
    Wq = np.asarray(Wq, dtype=np.float32)
    Wk = np.asarray(Wk, dtype=np.float32)
    Wv = np.asarray(Wv, dtype=np.float32)
    Wo = np.asarray(Wo, dtype=np.float32)
    bq = np.asarray(bq, dtype=np.float32)
    bk = np.asarray(bk, dtype=np.float32)
    bv = np.asarray(bv, dtype=np.float32)



    def wbf_layout(W, hs):
        wt = W[hs].T.astype(BF)                       # [1024, 256]
        return np.ascontiguousarray(
            wt.reshape(DM, 128, E).transpose(1, 0, 2))
    def wf8_layout(W, hs):
        wt = (W[hs].T * WS).astype(E4)
        return np.ascontiguousarray(
            wt.reshape(DM // 2, 2, 128, E).transpose(2, 0, 1, 3))

    in_maps = []
    for c in range(NCORES):
        b, g = divmod(c, 4)
        hs = slice(g * E, (g + 1) * E)
        xT = np.ascontiguousarray(x[b].T)             # [1024, 2048]
        x4 = xT.reshape(DM, 128, T).transpose(1, 0, 2)  # [128, 8, 2048]
        cf32 = np.stack([bq[hs][0:128], bq[hs][128:256],
                         bk[hs][0:128], bk[hs][128:256]], axis=1)
        cbf = np.zeros((33, 128 + E), dtype=BF)
        cbf[:, 0:128] = 1.0
        cbf[0, 128:] = bv[hs].astype(BF)
        cf8 = np.zeros((1, 128 + E), dtype=E4)
        cf8[:, 0:128] = 1.0
        cf8[0, 128:] = (bv[hs] * WS).astype(E4)
        in_maps.append({
            "xbf": np.ascontiguousarray(x4[:, :, 0:TQ]).astype(BF),
            "xf8": np.ascontiguousarray(x4[:, :, TQ:]).astype(E4),
            "wqbf": wbf_layout(Wq, hs), "wqf8": wf8_layout(Wq, hs),
            "wkbf": wbf_layout(Wk, hs), "wkf8": wf8_layout(Wk, hs),
            "wvbf": wbf_layout(Wv, hs), "wvf8": wf8_layout(Wv, hs),
            "wo": np.ascontiguousarray(Wo[:, hs].T).astype(BF),
            "cf32": np.ascontiguousarray(cf32, dtype=np.float32),
            "cbf": cbf, "cf8": cf8,
        })
    return in_maps


def kernel(x, Wq, bq, Wk, bk, Wv, bv, Wo, bo, _run_kwargs=None):
    nc = _get_nc()
    in_maps = _make_in_maps(x, Wq, bq, Wk, bk, Wv, bv, Wo, bo)
    last_err = None
    for _attempt in range(3):
        try:
            res = run_bass_kernel_spmd(nc, in_maps, core_ids=list(range(NCORES)),
                                       **(_run_kwargs or {}))
            break
        except Exception as e:  # transient NRT/device hiccups: retry
            last_err = e
            import time as _time
            _time.sleep(2.0)
    else:
        raise last_err
    bo = np.asarray(bo, dtype=np.float32)
    out = np.empty((B, T, D), dtype=np.float32)
    for b in range(B):
        acc = res.results[4 * b]["outT"].astype(np.float32)
        for g in range(1, 4):
            acc += res.results[4 * b + g]["outT"].astype(np.float32)
        out[b] = acc.T + bo
    if _run_kwargs:
        _CACHE['last_results'] = res
    return out
